# revision 16
# baseline (speedup 1.0000x reference)
"""Trainium2 Bass kernel for BinaryLinear: y = x @ sign(weight).T

Full shapes: x [32, 4096, 1024] f32, weight [1024, 1024] f32 -> y [32, 4096, 1024] f32.
Sharding: data-parallel over tokens across 8 cores (16384 tokens each); the
small weight is replicated and binarized + transposed on-chip per core.

Per-core pipeline (128-token tiles):
  1. SWDGE cast-load: x tile [128 t, 1024 i] f32 (HBM) -> f16 (SBUF)
  2. xbar DMA transpose: -> xT [128 i_inner, 8 i_chunk, 128 t] f16
  3. 16 matmuls (N=512, f16 in / f32 PSUM accum): psum[t, o] += xT.T @ Wsign^T
  4. copy PSUM -> SBUF f32, DMA out.
"""

import os
from contextlib import ExitStack

import numpy as np

import concourse.bass as bass
import concourse.mybir as mybir
import concourse.tile as tile
from concourse import bacc
from concourse.bass import ts
from concourse.bass_utils import run_bass_kernel_spmd

P = 128
N_CORES = 8
F32 = mybir.dt.float32
F16 = mybir.dt.float16

FULL_B, FULL_S, D_IN = 32, 4096, 1024
D_OUT = 1024
TOKENS_PER_CORE = FULL_B * FULL_S // N_CORES  # 16384


def build_nc(tokens=TOKENS_PER_CORE, d_in=D_IN, d_out=D_OUT):
    """Build the per-core Bass program: y[t,o] = sum_i x[t,i] * sign(w)[o,i]."""
    assert tokens % P == 0 and d_in % P == 0 and d_out % 512 == 0
    k_ch = d_in // P    # contraction chunks of 128
    o_ch = d_out // P   # weight row chunks of 128
    t_tiles = tokens // P

    nc = bacc.Bacc("TRN2")
    x = nc.dram_tensor("x", [tokens, d_in], F32, kind="ExternalInput")
    w = nc.dram_tensor("w", [d_out, d_in], F32, kind="ExternalInput")
    y = nc.dram_tensor("y", [tokens, d_out], F32, kind="ExternalOutput")

    TG = 4 if t_tiles % 4 == 0 else 2  # 128-token tiles per load/transpose batch
    SG = 2                             # 128-token tiles per store batch
    n_groups = t_tiles // TG
    PF = min(3, n_groups)              # prefetch depth (groups)
    n_halves = d_out // 512

    with tile.TileContext(nc) as tc, ExitStack() as ctx:
        xpool = ctx.enter_context(tc.tile_pool(name="xin", bufs=3))
        x16pool = ctx.enter_context(tc.tile_pool(name="x16", bufs=3))
        xTpool = ctx.enter_context(tc.tile_pool(name="xT", bufs=3))
        pspool = ctx.enter_context(tc.tile_pool(name="ps", bufs=4, space="PSUM"))
        opool = ctx.enter_context(tc.tile_pool(name="out", bufs=5))
        wpool = ctx.enter_context(tc.tile_pool(name="wprep", bufs=2))
        rpool = ctx.enter_context(tc.tile_pool(name="rhs", bufs=1))

        x_g = x.rearrange("(g a p) i -> g p a i", p=P, a=TG)
        y_g = y.rearrange("(h a p) o -> h p a o", p=P, a=SG)

        xins = {}
        xTs = {}

        def emit_load(g):
            xin = xpool.tile([P, TG, d_in], F32, name="xin")
            nc.gpsimd.dma_start(xin, x_g[g])
            xins[g] = xin

        def emit_xform(g):
            xin = xins.pop(g)
            x16 = x16pool.tile([P, TG * d_in], F16, name="x16")
            nc.gpsimd.tensor_copy(x16, xin.rearrange("p a i -> p (a i)"))  # cast
            xT = xTpool.tile([P, TG * k_ch, P], F16, name="xT")
            nc.sync.dma_start_transpose(xT, x16)
            xTs[g] = xT

        XF = min(2, n_groups)  # xform prefetch depth (< PF load depth)

        # ---- one-time weight prep: Rh[nh][i_inner, i_chunk, o_half] ----
        # Split over out-halves so the first matmuls depend on only half of it.
        Rh = [
            rpool.tile([P, k_ch, d_out // 2], F16, name=f"R{h}")
            for h in range(2)
        ]
        for c in range(o_ch):
            wt = wpool.tile([P, d_in], F32, name="wt", tag="wt")
            nc.scalar.dma_start(wt, w[ts(c, P), :])
            s16 = wpool.tile([P, d_in], F16, name="s16", tag="s16")
            nc.scalar.activation(s16, wt, mybir.ActivationFunctionType.Sign)
            wtmp = wpool.tile([P, k_ch, P], F16, name="wtmp", tag="wtmp")
            nc.sync.dma_start_transpose(wtmp, s16)
            half, co = divmod(c, o_ch // 2)
            nc.vector.tensor_copy(Rh[half][:, :, ts(co, P)], wtmp)

        # ---- prologue: fill the x pipeline ----
        emit_load(0)
        emit_xform(0)
        for g in range(1, PF):
            emit_load(g)
        if n_groups > 1:
            emit_xform(1)

        # ---- main loop ----
        out = None
        for g in range(n_groups):
            xT = xTs.pop(g)
            for a in range(TG):
                t_idx = g * TG + a          # global 128-token tile index
                sa = t_idx % SG
                if sa == 0:
                    out = opool.tile([P, SG, d_out], F32, name="out")
                ps = pspool.tile([P, d_out], F32, name="ps")
                for nh in range(n_halves):
                    for k in range(k_ch):
                        nc.tensor.matmul(
                            ps[:, ts(nh, 512)],
                            xT[:, a * k_ch + k, :],
                            Rh[nh][:, k, :],
                            start=(k == 0),
                            stop=(k == k_ch - 1),
                        )
                if a % 2 == 0:
                    nc.vector.tensor_copy(out[:, sa, :], ps)
                else:
                    nc.scalar.copy(out[:, sa, :], ps)
                if sa == SG - 1:
                    nc.scalar.dma_start(y_g[t_idx // SG], out)
            if g + XF < n_groups:
                emit_xform(g + XF)
            if g + PF < n_groups:
                emit_load(g + PF)
    nc.compile()
    return nc


_NC_CACHE = {}


def _get_nc():
    key = (TOKENS_PER_CORE, D_IN, D_OUT)
    if key not in _NC_CACHE:
        _NC_CACHE[key] = build_nc()
    return _NC_CACHE[key]


def run(x, weight, trace=False, **kwargs):
    """Shard, execute on 8 cores, gather. Returns (y_full, BassKernelResults)."""
    x = np.ascontiguousarray(x, dtype=np.float32)
    weight = np.ascontiguousarray(weight, dtype=np.float32)
    assert x.shape == (FULL_B, FULL_S, D_IN), x.shape
    assert weight.shape == (D_OUT, D_IN), weight.shape

    x_flat = x.reshape(FULL_B * FULL_S, D_IN)
    shards = x_flat.reshape(N_CORES, TOKENS_PER_CORE, D_IN)
    in_maps = [{"x": shards[c], "w": weight} for c in range(N_CORES)]

    nc = _get_nc()
    res = run_bass_kernel_spmd(
        nc, in_maps, core_ids=list(range(N_CORES)), trace=trace, **kwargs
    )
    y = np.concatenate([res.results[c]["y"] for c in range(N_CORES)], axis=0)
    return y.reshape(FULL_B, FULL_S, D_OUT), res


def kernel(x, weight):
    y, _ = run(x, weight)
    return y


# revision 17
# speedup vs baseline: 1.2449x; 1.2449x over previous
"""Trainium2 Bass kernel for BinaryLinear: y = x @ sign(weight).T

Full shapes: x [32, 4096, 1024] f32, weight [1024, 1024] f32 -> y [32, 4096, 1024] f32.
Sharding: data-parallel over tokens across 8 cores (16384 tokens each); the
small weight is replicated and binarized + transposed on-chip per core.

Per-core pipeline (128-token tiles):
  1. SWDGE cast-load: x tile [128 t, 1024 i] f32 (HBM) -> f16 (SBUF)
  2. xbar DMA transpose: -> xT [128 i_inner, 8 i_chunk, 128 t] f16
  3. 16 matmuls (N=512, f16 in / f32 PSUM accum): psum[t, o] += xT.T @ Wsign^T
  4. copy PSUM -> SBUF f32, DMA out.
"""

import os
from contextlib import ExitStack

import numpy as np

import concourse.bass as bass
import concourse.mybir as mybir
import concourse.tile as tile
from concourse import bacc
from concourse.bass import ts
from concourse.bass_utils import run_bass_kernel_spmd

P = 128
N_CORES = 8
F32 = mybir.dt.float32
F16 = mybir.dt.float16

FULL_B, FULL_S, D_IN = 32, 4096, 1024
D_OUT = 1024
TOKENS_PER_CORE = FULL_B * FULL_S // N_CORES  # 16384


def build_nc(tokens=TOKENS_PER_CORE, d_in=D_IN, d_out=D_OUT):
    """Build the per-core Bass program: y[t,o] = sum_i x[t,i] * sign(w)[o,i]."""
    assert tokens % P == 0 and d_in % P == 0 and d_out % 512 == 0
    k_ch = d_in // P    # contraction chunks of 128
    o_ch = d_out // P   # weight row chunks of 128
    t_tiles = tokens // P

    nc = bacc.Bacc("TRN2")
    x = nc.dram_tensor("x", [tokens, d_in], F32, kind="ExternalInput")
    w = nc.dram_tensor("w", [d_out, d_in], F32, kind="ExternalInput")
    y = nc.dram_tensor("y", [tokens, d_out], F32, kind="ExternalOutput")

    TG = 4 if t_tiles % 4 == 0 else 2  # 128-token tiles per load/transpose batch
    SG = 2                             # 128-token tiles per store batch
    n_groups = t_tiles // TG
    PF = min(3, n_groups)              # prefetch depth (groups)
    n_halves = d_out // 512

    with tile.TileContext(nc) as tc, ExitStack() as ctx:
        xpool = ctx.enter_context(tc.tile_pool(name="xin", bufs=3))
        x16pool = ctx.enter_context(tc.tile_pool(name="x16", bufs=3))
        xTpool = ctx.enter_context(tc.tile_pool(name="xT", bufs=3))
        pspool = ctx.enter_context(tc.tile_pool(name="ps", bufs=4, space="PSUM"))
        opool = ctx.enter_context(tc.tile_pool(name="out", bufs=5))
        wpool = ctx.enter_context(tc.tile_pool(name="wprep", bufs=2))
        rpool = ctx.enter_context(tc.tile_pool(name="rhs", bufs=1))

        x_g = x.rearrange("(g a p) i -> g p a i", p=P, a=TG)
        y_g = y.rearrange("(h a p) o -> h p a o", p=P, a=SG)

        xins = {}
        xTs = {}

        def emit_load(g):
            xin = xpool.tile([P, TG, d_in], F32, name="xin")
            nc.gpsimd.dma_start(xin, x_g[g])
            xins[g] = xin

        def emit_xform(g):
            xin = xins.pop(g)
            x16 = x16pool.tile([P, TG * d_in], F16, name="x16")
            nc.vector.tensor_copy(x16, xin.rearrange("p a i -> p (a i)"))  # cast
            xT = xTpool.tile([P, TG * k_ch, P], F16, name="xT")
            nc.sync.dma_start_transpose(xT, x16)
            xTs[g] = xT

        XF = min(2, n_groups)  # xform prefetch depth (< PF load depth)

        # ---- one-time weight prep: Rh[nh][i_inner, i_chunk, o_half] ----
        # Split over out-halves so the first matmuls depend on only half of it.
        Rh = [
            rpool.tile([P, k_ch, d_out // 2], F16, name=f"R{h}")
            for h in range(2)
        ]
        for c in range(o_ch):
            wt = wpool.tile([P, d_in], F32, name="wt", tag="wt")
            nc.scalar.dma_start(wt, w[ts(c, P), :])
            s16 = wpool.tile([P, d_in], F16, name="s16", tag="s16")
            nc.scalar.activation(s16, wt, mybir.ActivationFunctionType.Sign)
            wtmp = wpool.tile([P, k_ch, P], F16, name="wtmp", tag="wtmp")
            nc.sync.dma_start_transpose(wtmp, s16)
            half, co = divmod(c, o_ch // 2)
            nc.vector.tensor_copy(Rh[half][:, :, ts(co, P)], wtmp)

        # ---- prologue: fill the x pipeline ----
        emit_load(0)
        emit_xform(0)
        for g in range(1, PF):
            emit_load(g)
        if n_groups > 1:
            emit_xform(1)

        # ---- main loop ----
        out = None
        for g in range(n_groups):
            xT = xTs.pop(g)
            for a in range(TG):
                t_idx = g * TG + a          # global 128-token tile index
                sa = t_idx % SG
                if sa == 0:
                    out = opool.tile([P, SG, d_out], F32, name="out")
                ps = pspool.tile([P, d_out], F32, name="ps")
                for nh in range(n_halves):
                    for k in range(k_ch):
                        nc.tensor.matmul(
                            ps[:, ts(nh, 512)],
                            xT[:, a * k_ch + k, :],
                            Rh[nh][:, k, :],
                            start=(k == 0),
                            stop=(k == k_ch - 1),
                        )
                if a % 2 == 0:
                    nc.vector.tensor_copy(out[:, sa, :], ps)
                else:
                    nc.scalar.copy(out[:, sa, :], ps)
                if sa == SG - 1:
                    nc.scalar.dma_start(y_g[t_idx // SG], out)
            if g + XF < n_groups:
                emit_xform(g + XF)
            if g + PF < n_groups:
                emit_load(g + PF)
    nc.compile()
    return nc


_NC_CACHE = {}


def _get_nc():
    key = (TOKENS_PER_CORE, D_IN, D_OUT)
    if key not in _NC_CACHE:
        _NC_CACHE[key] = build_nc()
    return _NC_CACHE[key]


def run(x, weight, trace=False, **kwargs):
    """Shard, execute on 8 cores, gather. Returns (y_full, BassKernelResults)."""
    x = np.ascontiguousarray(x, dtype=np.float32)
    weight = np.ascontiguousarray(weight, dtype=np.float32)
    assert x.shape == (FULL_B, FULL_S, D_IN), x.shape
    assert weight.shape == (D_OUT, D_IN), weight.shape

    x_flat = x.reshape(FULL_B * FULL_S, D_IN)
    shards = x_flat.reshape(N_CORES, TOKENS_PER_CORE, D_IN)
    in_maps = [{"x": shards[c], "w": weight} for c in range(N_CORES)]

    nc = _get_nc()
    res = run_bass_kernel_spmd(
        nc, in_maps, core_ids=list(range(N_CORES)), trace=trace, **kwargs
    )
    y = np.concatenate([res.results[c]["y"] for c in range(N_CORES)], axis=0)
    return y.reshape(FULL_B, FULL_S, D_OUT), res


def kernel(x, weight):
    y, _ = run(x, weight)
    return y
